# revision 3
# baseline (speedup 1.0000x reference)
"""ChannelDeconv (whitening) kernel for 8 Trainium2 NeuronCores.

Math (matches the reference):
  x1  = x.transpose(1,0,2,3).reshape(64, N*H*W)
  x1s = x1[:, ::9]
  mean = x1s.mean(axis=-1);  cov = x1s @ x1s.T / x1s.shape[1] + 0.01*I
  D = newton_schulz_isqrt(cov, 5);  out = D @ (x1 - mean)

Distribution: columns of x1 are split into 8 shards whose start offsets are
multiples of 9 (so the stride-9 subsample is phase-0 on every core and one
SPMD program serves all cores), zero-padded to a common width.  Each core
computes partial cov/mean sums, a 64x65 AllReduce combines them, Newton-
Schulz runs replicated, and each core applies the deconv to its shard.

Device pipeline per core:
  stats:  DMA [64, 4608] tiles -> strided PE transpose of the stride-9
          subsample into [128, 64] chunks -> PSUM-accumulated X^T X matmuls
          (cov) + strided DVE reduction (mean sums)
  AR:     16.6 KB AllReduce over the 8 cores (cov sums + mean sums)
  NS:     Frobenius norm + 5 Newton-Schulz iterations on 64x64 tiles
          (all iterates are symmetric, so lhsT = M stands in for M^T)
  apply:  out = D @ x - (D @ mean) 1^T  as one fused matmul: the stationary
          operand is [D ; -(D@mean)^T] (65x64) and each moving tile carries
          an appended ones-row (65 partitions).
"""

import sys

import numpy as np

if "/opt/trn_rl_repo" not in sys.path:
    sys.path.insert(0, "/opt/trn_rl_repo")

import concourse.bacc as bacc
import concourse.tile as tile
from concourse import mybir
from concourse import bass_utils
from concourse.bass_interp import get_hw_module

FP32 = mybir.dt.float32

C = 64
N_CORES = 8
EPS = 0.01
N_ITER = 5
SS = 9  # stride**2

STATS_TILE = 4608  # multiple of 9: every stats tile has width % 9 == 0
APPLY_TILE = 2048
MM_N = 512  # fp32 moving-operand / PSUM-bank limit


def shard_plan(total_cols: int, n_cores: int = N_CORES):
    base = (total_cols // n_cores) // SS * SS
    starts = [k * base for k in range(n_cores)]
    widths = [base] * (n_cores - 1) + [total_cols - (n_cores - 1) * base]
    padded = -(-max(widths) // SS) * SS
    return starts, widths, padded


def build_program(wp: int, total_count: int, n_cores: int = N_CORES):
    """Build + compile the SPMD Bass program for per-core padded width wp."""
    assert wp % SS == 0
    nc = bacc.Bacc(
        "TRN2", target_bir_lowering=False, debug=False, num_devices=n_cores
    )
    xs = nc.dram_tensor("xs", [C, wp], FP32, kind="ExternalInput").ap()
    out = nc.dram_tensor("out", [C, wp], FP32, kind="ExternalOutput").ap()

    eye_np = np.eye(C, dtype=np.float32)
    ident_h = nc.inline_tensor(eye_np, name="ident64")
    eye15_h = nc.inline_tensor(np.float32(1.5) * eye_np, name="eye15")
    epseye_h = nc.inline_tensor(np.float32(EPS) * eye_np, name="epseye")
    onescol_h = nc.inline_tensor(np.ones((C, 1), np.float32), name="onescol")
    onesrow_h = nc.inline_tensor(np.ones((1, C), np.float32), name="onesrow")

    ar_in = nc.dram_tensor("ar_in", [C, C + 1], FP32, kind="Internal")
    ar_out = nc.dram_tensor(
        "ar_out", [C, C + 1], FP32, kind="Internal", addr_space="Shared"
    )

    stats_tiles = []
    off = 0
    while off < wp:
        tw = min(STATS_TILE, wp - off)
        assert tw % SS == 0
        stats_tiles.append((off, tw))
        off += tw

    apply_tiles = []
    off = 0
    while off < wp:
        tw = min(APPLY_TILE, wp - off)
        apply_tiles.append((off, tw))
        off += tw

    inv_count = float(np.float32(1.0) / np.float32(total_count))

    with tile.TileContext(nc) as tc:
        with tc.tile_pool(name="singles", bufs=1) as singles:
            ident_sb = singles.tile([C, C], FP32)
            nc.sync.dma_start(out=ident_sb, in_=ident_h.ap())
            eye15_sb = singles.tile([C, C], FP32)
            nc.sync.dma_start(out=eye15_sb, in_=eye15_h.ap())
            epseye_sb = singles.tile([C, C], FP32)
            nc.sync.dma_start(out=epseye_sb, in_=epseye_h.ap())
            onescol_sb = singles.tile([C, 1], FP32)
            nc.sync.dma_start(out=onescol_sb, in_=onescol_h.ap())
            onesrow_sb = singles.tile([1, C], FP32)
            nc.sync.dma_start(out=onesrow_sb, in_=onesrow_h.ap())

            cov_acc = singles.tile([C, C], FP32)
            nc.vector.memset(cov_acc, 0.0)
            macc_cols = len(stats_tiles)
            macc = singles.tile([C, macc_cols], FP32)
            nc.vector.memset(macc, 0.0)

            # ---------------- stats ----------------
            with (
                tc.tile_pool(name="sx", bufs=3) as sx_pool,
                tc.tile_pool(name="str", bufs=4) as str_pool,
                tc.tile_pool(name="ptr", bufs=4, space="PSUM") as ptr_pool,
                tc.tile_pool(name="pcov", bufs=2, space="PSUM") as pcov_pool,
            ):
                for ti, (off, tw) in enumerate(stats_tiles):
                    sub = tw // SS
                    xt = sx_pool.tile([C, STATS_TILE], FP32, tag="xt")
                    nc.sync.dma_start(out=xt[:, :tw], in_=xs[:, off : off + tw])
                    subv = xt[:, : sub * SS : SS]  # [C, sub], stride 9
                    nc.vector.reduce_sum(
                        out=macc[:, ti : ti + 1],
                        in_=subv,
                        axis=mybir.AxisListType.X,
                    )
                    covp = pcov_pool.tile([C, C], FP32, tag="covp")
                    nchunks = -(-sub // 128)
                    for ci in range(nchunks):
                        c0 = ci * 128
                        cw = min(128, sub - c0)
                        src = xt[:, c0 * SS : (c0 + cw) * SS : SS]  # [C, cw]
                        ptr = ptr_pool.tile([128, C], FP32, tag="ptr")
                        nc.tensor.transpose(ptr[:cw, :], src, ident_sb)
                        xtr = str_pool.tile([128, C], FP32, tag="xtr")
                        nc.scalar.copy(xtr[:cw, :], ptr[:cw, :])
                        nc.tensor.matmul(
                            covp,
                            lhsT=xtr[:cw, :],
                            rhs=xtr[:cw, :],
                            start=(ci == 0),
                            stop=(ci == nchunks - 1),
                        )
                    nc.vector.tensor_add(cov_acc, cov_acc, covp)

            # ---------------- all-reduce ----------------
            msum = singles.tile([C, 1], FP32)
            nc.vector.reduce_sum(out=msum, in_=macc, axis=mybir.AxisListType.X)
            nc.gpsimd.dma_start(out=ar_in[:, 0:C], in_=cov_acc)
            nc.gpsimd.dma_start(out=ar_in[:, C : C + 1], in_=msum)
            nc.gpsimd.collective_compute(
                "AllReduce",
                mybir.AluOpType.add,
                replica_groups=[list(range(n_cores))],
                ins=[ar_in.ap()],
                outs=[ar_out.ap()],
            )
            red = singles.tile([C, C + 1], FP32)
            nc.sync.dma_start(out=red, in_=ar_out.ap())

            # ---------------- newton-schulz (replicated) ----------------
            covf = singles.tile([C, C], FP32)
            nc.vector.tensor_scalar_mul(covf, red[:, 0:C], inv_count)
            nc.vector.tensor_add(covf, covf, epseye_sb)
            meanf = singles.tile([C, 1], FP32)
            nc.vector.tensor_scalar_mul(meanf, red[:, C : C + 1], inv_count)

            sq = singles.tile([C, C], FP32)
            nc.vector.tensor_mul(sq, covf, covf)
            rs = singles.tile([C, 1], FP32)
            nc.vector.reduce_sum(out=rs, in_=sq, axis=mybir.AxisListType.X)

            lhsT_apply = singles.tile([C + 1, C], FP32)

            with (
                tc.tile_pool(name="pns", bufs=3, space="PSUM") as pns,
                tc.tile_pool(name="nsw", bufs=3) as nsw,
            ):
                f2p = pns.tile([1, 1], FP32, tag="p")
                nc.tensor.matmul(
                    f2p, lhsT=onescol_sb, rhs=rs, start=True, stop=True
                )
                # sc = [normA, 1/normA, sqrt(normA), 1/sqrt(normA)] on part. 0
                sc = singles.tile([1, 4], FP32)
                nc.scalar.sqrt(sc[:, 0:1], f2p)
                nc.vector.reciprocal(sc[:, 1:2], sc[:, 0:1])
                nc.scalar.sqrt(sc[:, 2:3], sc[:, 0:1])
                nc.vector.reciprocal(sc[:, 3:4], sc[:, 2:3])
                bcp = pns.tile([C, 2], FP32, tag="p")
                nc.tensor.matmul(
                    bcp, lhsT=onesrow_sb, rhs=sc[:, 1:4:2], start=True, stop=True
                )
                bc = singles.tile([C, 2], FP32)  # [1/normA, 1/sqrt(normA)]
                nc.scalar.copy(bc, bcp)

                y = nsw.tile([C, C], FP32, tag="Y", name="y0")
                nc.vector.tensor_scalar_mul(y, covf, bc[:, 0:1])
                # iteration 1 with Z0 = I folded away
                t = nsw.tile([C, C], FP32, tag="T", name="t1")
                nc.scalar.mul(t, y, -0.5)
                nc.vector.tensor_add(t, t, eye15_sb)
                p2 = pns.tile([C, C], FP32, tag="p")
                nc.tensor.matmul(p2, lhsT=y, rhs=t, start=True, stop=True)
                ynew = nsw.tile([C, C], FP32, tag="Y", name="y1")
                nc.scalar.copy(ynew, p2)
                z, y = t, ynew
                for it in range(N_ITER - 1):
                    p1 = pns.tile([C, C], FP32, tag="p", name=f"pzy{it}")
                    nc.tensor.matmul(p1, lhsT=z, rhs=y, start=True, stop=True)
                    t = nsw.tile([C, C], FP32, tag="T", name=f"t{it}")
                    nc.scalar.mul(t, p1, -0.5)
                    nc.vector.tensor_add(t, t, eye15_sb)
                    p2 = pns.tile([C, C], FP32, tag="p", name=f"pyt{it}")
                    nc.tensor.matmul(p2, lhsT=y, rhs=t, start=True, stop=True)
                    p3 = pns.tile([C, C], FP32, tag="p", name=f"ptz{it}")
                    nc.tensor.matmul(p3, lhsT=t, rhs=z, start=True, stop=True)
                    ynew = nsw.tile([C, C], FP32, tag="Y", name=f"y{it + 2}")
                    nc.scalar.copy(ynew, p2)
                    znew = nsw.tile([C, C], FP32, tag="Z", name=f"z{it + 2}")
                    nc.scalar.copy(znew, p3)
                    y, z = ynew, znew

                # stationary operand: rows 0..63 = D, row 64 = -(D @ mean)^T
                nc.vector.tensor_scalar_mul(lhsT_apply[0:C, :], z, bc[:, 1:2])
                pdm = pns.tile([1, C], FP32, tag="p")
                nc.tensor.matmul(
                    pdm, lhsT=meanf, rhs=lhsT_apply[0:C, :], start=True, stop=True
                )
                nc.scalar.mul(lhsT_apply[C : C + 1, :], pdm, -1.0)

            # ---------------- apply ----------------
            with (
                tc.tile_pool(name="rtp", bufs=1) as rt_pool,
                tc.tile_pool(name="otp", bufs=3) as ot_pool,
                tc.tile_pool(name="pap", bufs=8, space="PSUM") as pap,
            ):
                rts = []
                for i in range(3):
                    rt = rt_pool.tile(
                        [C + 1, APPLY_TILE], FP32, tag=f"rt{i}", name=f"rt{i}"
                    )
                    nc.vector.memset(rt[C : C + 1, :], 1.0)
                    rts.append(rt)
                for i, (off, tw) in enumerate(apply_tiles):
                    rt = rts[i % 3]
                    nc.sync.dma_start(out=rt[0:C, :tw], in_=xs[:, off : off + tw])
                    ot = ot_pool.tile([C, APPLY_TILE], FP32, tag="ot")
                    for s in range(-(-tw // MM_N)):
                        w = min(MM_N, tw - s * MM_N)
                        ps = pap.tile([C, MM_N], FP32, tag="ps")
                        nc.tensor.matmul(
                            ps[:, :w],
                            lhsT=lhsT_apply,
                            rhs=rt[:, s * MM_N : s * MM_N + w],
                            start=True,
                            stop=True,
                        )
                        if s % 2 == 0:
                            nc.scalar.copy(ot[:, s * MM_N : s * MM_N + w], ps[:, :w])
                        else:
                            nc.vector.tensor_copy(
                                ot[:, s * MM_N : s * MM_N + w], ps[:, :w]
                            )
                    nc.sync.dma_start(out=out[:, off : off + tw], in_=ot[:, :tw])

    nc.compile()
    return nc


_PROGRAM_CACHE: dict = {}

# test-harness knobs (harness calls kernel() directly with these defaults)
TRACE = False
LAST_RESULTS = None


def _get_program(wp: int, total_count: int):
    key = (wp, total_count)
    if key not in _PROGRAM_CACHE:
        _PROGRAM_CACHE[key] = build_program(wp, total_count)
    return _PROGRAM_CACHE[key]


def kernel(x: np.ndarray) -> np.ndarray:
    x = np.asarray(x)
    n, c, h, w = x.shape
    assert c == C
    total = n * h * w
    x1 = np.ascontiguousarray(x.transpose(1, 0, 2, 3).reshape(C, total))
    starts, widths, wp = shard_plan(total)
    total_count = -(-total // SS)

    nc = _get_program(wp, total_count)

    in_maps = []
    for k in range(N_CORES):
        sh = np.zeros((C, wp), np.float32)
        sh[:, : widths[k]] = x1[:, starts[k] : starts[k] + widths[k]]
        in_maps.append({"xs": sh})

    global LAST_RESULTS
    old_m = nc.m
    nc.m = get_hw_module(nc.m)
    try:
        res = bass_utils.run_bass_kernel_spmd(
            nc, in_maps, core_ids=list(range(N_CORES)), trace=TRACE
        )
    finally:
        nc.m = old_m
    LAST_RESULTS = res

    out1 = np.empty((C, total), np.float32)
    for k in range(N_CORES):
        out1[:, starts[k] : starts[k] + widths[k]] = res.results[k]["out"][
            :, : widths[k]
        ]
    return np.ascontiguousarray(out1.reshape(C, n, h, w).transpose(1, 0, 2, 3))
